# revision 29
# baseline (speedup 1.0000x reference)
"""Dynamic-kernel CNN (conv5x5->tanh gate->windowed sum) on 8 trn2 cores.

out(y,x) = sum_{dx,dy} xq[y+dy, x+dx] * tanh( sum_{k} W2[c,k] V_k + b_c ),
with xq = pad2(x) [32x32], c = k = 5*dx+dy, V_k(y,x) = xq[y+dy, x+dx].

Data-parallel over batch: 2048 images -> 256 per core.

Per-core layout: partitions = (strip s in 0..4) x (channel k in 0..24) = 125.
Strip s of group g handles image 125*r + 5*j + s  (g = 25*r + j).
Free dim = pixel plane (28*28 = 784), stored at row pitch 32.

v3: bf16 compute + three-hop batched im2col gather (few big DMAs).
  DMA instructions have ~2us fixed cost each; the old per-group gather
  (260 DMAs) serialized ~560us on the Sync engine. DMA in-APs may only
  cross partitions in their FIRST level, so the im2col is staged:
  - XQ2 (per r,s: 15 DMAs): XQ2[5r+s, 1024*j + u] = xq[5j+s, 1024r+u]
    -- images of one strip concatenated along the free dim.
  - Z (per r,dy: 15 DMAs): Z[5dy+s, 896*j + t] = XQ2[5r+s, 1024j+32dy+t]
    -- dy-shifted row-windows, groups re-periodized to EXACTLY 896.
  - V (per chunk of 5 groups, ONE DMA): V[25dy+5s+dx, 896j + w] =
    Z[5dy+s, 896j + w + dx]. In-AP [[pitch,25],[1,5],[1,896*csize]]:
    (dy,s) are consecutive partitions and windows are 896-periodic.
    The dx shift bleeds 4 cross-window elements into w in {892..895}
    = (y=27, x>=28) pad columns, which no consumer reads.

Pipeline per group of 5 images:
  1. V [125, 784-of-896] slice of the chunk gather.
  2. FC = blockdiag(W2^T)^T @ V  (two bf16 matmuls into one PSUM tile)
  3. G = tanh(FC + b) on ACT (bias fused, PSUM -> SBUF bf16)
  4. M = V * G elementwise (DVE, bf16)
  5. per-strip channel sum: bf16 matmul with ones-blockdiag placed at
     output partitions 5j..5j+4, accumulating 25 groups into PSUM fp32.
  6. per round of 25 groups: evacuate PSUM -> SBUF -> DMA to output.
"""

import numpy as np
from contextlib import ExitStack

import ml_dtypes

import concourse.bass as bass
import concourse.tile as tile
from concourse import bacc, mybir
from concourse import bass_utils

F32 = mybir.dt.float32
BF16 = mybir.dt.bfloat16
TANH = mybir.ActivationFunctionType.Tanh

N_CORES = 8
B_FULL = 2048
B_LOC = B_FULL // N_CORES  # 256
NPIX = 784                 # 28*28
XQ_LEN = 1024              # 32*32 padded plane
VW = 896                   # 28 rows x 32 cols: per-group window pitch
HALF = 392                 # half pixel plane
CHUNK = 5                  # groups per hop-2 gather DMA
RED_DELAY = 2              # groups of lag for the reduce matmuls

# image -> (round r, group j, strip s): img = 125*r + 5*j + s
ROUNDS = ((0, 25, 125), (1, 25, 125), (2, 2, 6))  # (r, n_groups, rows stored)
MAXG = 25                  # max groups per round


def _emit(ctx, tc, x_d, wblk_d, mbig_d, bias_d, y_d):
    nc = tc.nc

    cpool = ctx.enter_context(tc.tile_pool(name="const", bufs=1))
    zpool = ctx.enter_context(tc.tile_pool(name="z", bufs=2))
    vpool = ctx.enter_context(tc.tile_pool(name="v", bufs=3))
    gpool = ctx.enter_context(tc.tile_pool(name="g", bufs=3))
    mpool = ctx.enter_context(tc.tile_pool(name="m", bufs=4))
    epool = ctx.enter_context(tc.tile_pool(name="evac", bufs=3))
    pfc = ctx.enter_context(tc.tile_pool(name="pfc", bufs=3, space="PSUM"))
    pred = ctx.enter_context(tc.tile_pool(name="pred", bufs=1, space="PSUM"))

    wblk = cpool.tile([125, 125], BF16)
    nc.sync.dma_start(wblk[:], wblk_d[:])
    mbig = cpool.tile([125, 245], BF16)
    nc.sync.dma_start(mbig[:], mbig_d[:])
    biasv = cpool.tile([125, 1], F32)
    nc.sync.dma_start(biasv[:], bias_d[:])

    # padded images: partition p holds xq of image 125*r + p at cols r*1024.
    xq = cpool.tile([128, 3 * XQ_LEN], BF16)
    stage = cpool.tile([128, 3 * NPIX], F32)
    nc.gpsimd.memset(xq[:], 0.0)
    for r, _, rows in ROUNDS:
        nc.sync.dma_start(
            stage[0:rows, NPIX * r : NPIX * (r + 1)],
            x_d[125 * r : 125 * r + rows, :],
        )
        src = stage[0:rows, NPIX * r : NPIX * (r + 1)].rearrange(
            "p (y x) -> p y x", x=28
        )
        dst = xq[0:rows, XQ_LEN * r : XQ_LEN * (r + 1)].rearrange(
            "p (y x) -> p y x", x=32
        )[:, 2:30, 2:30]
        nc.gpsimd.tensor_copy(dst, src)

    xq_ap = xq[:]
    xq_pitch = xq_ap.ap[0][0]  # partition stride in elements

    # --- XQ2[5r+s, 1024*j + u] = xq[5j+s, 1024r + u] ---
    xq2 = cpool.tile([15, XQ_LEN * MAXG], BF16)
    xq2_ap = xq2[:]
    xq2_pitch = xq2_ap.ap[0][0]
    dma_eng = (nc.sync, nc.gpsimd)  # keep scalar's FIFO queue ACT-only
    for r, n_groups, rows in ROUNDS:
        for s in range(5):
            in_ap = bass.AP(
                tensor=xq_ap.tensor,
                offset=xq_ap.offset + xq_pitch * s + XQ_LEN * r,
                ap=[
                    [5 * xq_pitch, n_groups],  # j -> partition step 5
                    [1, XQ_LEN],               # u -> free
                ],
            )
            nc.gpsimd.dma_start(
                xq2[5 * r + s : 5 * r + s + 1, 0 : XQ_LEN * n_groups], in_ap
            )

    z_tiles = {}

    def emit_z(ri):
        # --- hop 1: Z[5dy+s, 896*j + t] = XQ2[5r+s, 1024j + 32dy + t] ---
        r, n_groups, _ = ROUNDS[ri]
        z = zpool.tile([25, VW * MAXG + 8], BF16, tag="z")
        z_tiles[ri] = z
        for dy in range(5):
            in_ap = bass.AP(
                tensor=xq2_ap.tensor,
                offset=xq2_ap.offset + xq2_pitch * 5 * r + 32 * dy,
                ap=[
                    [xq2_pitch, 5],        # s -> partition
                    [XQ_LEN, n_groups],    # j -> free block (in-partition)
                    [1, VW],               # t -> free
                ],
            )
            nc.sync.dma_start(
                z[5 * dy : 5 * dy + 5, 0 : VW * n_groups], in_ap
            )

    emit_z(0)
    for ri, (r, n_groups, rows) in enumerate(ROUNDS):
        z = z_tiles[ri]
        z_ap = z[:]
        z_pitch = z_ap.ap[0][0]

        red_a = pred.tile([125, HALF], F32, tag="red_a")
        red_b = pred.tile([125, HALF], F32, tag="red_b")

        # software pipeline: delay each group's reduce matmuls by RED_DELAY
        # groups so the strict-FIFO tensor queue never waits on the
        # ACT->DVE chain of the same group (keeps the PE warm, K=8/8).
        pend = []

        def emit_red(j, m):
            ones_j = mbig[:, 120 - 5 * j : 245 - 5 * j]
            nc.tensor.matmul(
                red_a[:], ones_j, m[:, 0:HALF],
                start=(j == 0), stop=(j == n_groups - 1),
                skip_group_check=True,
            )
            nc.tensor.matmul(
                red_b[:], ones_j, m[:, HALF:NPIX],
                start=(j == 0), stop=(j == n_groups - 1),
                skip_group_check=True,
            )

        for c0 in range(0, n_groups, CHUNK):
            csize = min(CHUNK, n_groups - c0)
            # --- hop 2: V[25dy+5s+dx, u] = Z[5dy+s, u+dx] ---
            # Split the 1.1 MB transfer across all 3 DMA rings; the SWDGE
            # (gpsimd) ring drains slowest, so it gets the smallest share.
            v = vpool.tile([125, VW * CHUNK], BF16, tag="v")
            for (p0, p1), eng in zip(
                ((0, 10), (10, 19), (19, 25)), (nc.sync, nc.scalar, nc.gpsimd)
            ):
                in_ap = bass.AP(
                    tensor=z_ap.tensor,
                    offset=z_ap.offset + z_pitch * p0 + VW * c0,
                    ap=[
                        [z_pitch, p1 - p0],  # (dy,s) -> partition blocks of 5
                        [1, 5],              # dx -> partition
                        [1, VW * csize],     # (j,w) -> free
                    ],
                )
                eng.dma_start(v[5 * p0 : 5 * p1, 0 : VW * csize], in_ap)

            # prefetch next round's Z mid-round so the pipeline never drains
            if c0 == 2 * CHUNK and ri + 1 < len(ROUNDS):
                emit_z(ri + 1)
            elif n_groups <= 2 * CHUNK and c0 == 0 and ri + 1 < len(ROUNDS):
                emit_z(ri + 1)

            for jj in range(csize):
                j = c0 + jj
                # strided views of the real 28x28 pixel plane
                vyx = v[:, VW * jj : VW * (jj + 1)].rearrange(
                    "p (y xc) -> p y xc", xc=32
                )[:, :, 0:28]

                # --- FC matmuls into one 2-bank PSUM tile ---
                fc = pfc.tile([125, 1024], F32)
                nc.tensor.matmul(
                    fc[:, 0:HALF], wblk[:], vyx[:, 0:14], start=True, stop=True
                )
                nc.tensor.matmul(
                    fc[:, 512 : 512 + HALF], wblk[:], vyx[:, 14:28],
                    start=True, stop=True,
                )

                # --- G = tanh(FC + b), one strided ACT over both banks ---
                g = gpool.tile([125, NPIX], BF16)
                fcv = fc[:].rearrange("p (t c) -> p t c", c=512)[:, :, 0:HALF]
                gv = g[:].rearrange("p (t c) -> p t c", c=HALF)
                nc.scalar.activation(gv, fcv, TANH, bias=biasv[:], scale=1.0)

                # --- M = V * G (DVE, 16-bit 2x rate) ---
                m = mpool.tile([125, NPIX], BF16)
                gyx = g[:].rearrange("p (y x) -> p y x", x=28)
                myx = m[:].rearrange("p (y x) -> p y x", x=28)
                nc.vector.tensor_mul(myx, vyx, gyx)

                pend.append((j, m))
                if len(pend) > RED_DELAY:
                    emit_red(*pend.pop(0))

        while pend:
            emit_red(*pend.pop(0))

        # --- evacuate + store round ---
        e_a = epool.tile([125, HALF], F32, tag="e_a")
        nc.vector.tensor_copy(e_a[:], red_a[:])
        nc.sync.dma_start(y_d[125 * r : 125 * r + rows, 0:HALF], e_a[0:rows, :])
        e_b = epool.tile([125, HALF], F32, tag="e_b")
        nc.vector.tensor_copy(e_b[:], red_b[:])
        nc.sync.dma_start(
            y_d[125 * r : 125 * r + rows, HALF:NPIX], e_b[0:rows, :]
        )


def build():
    nc = bacc.Bacc("TRN2", target_bir_lowering=False, debug=False)
    x_d = nc.dram_tensor("x", [B_LOC, NPIX], F32, kind="ExternalInput").ap()
    wblk_d = nc.dram_tensor("wblk", [125, 125], BF16, kind="ExternalInput").ap()
    mbig_d = nc.dram_tensor("mbig", [125, 245], BF16, kind="ExternalInput").ap()
    bias_d = nc.dram_tensor("biasv", [125, 1], F32, kind="ExternalInput").ap()
    y_d = nc.dram_tensor("y", [B_LOC, NPIX], F32, kind="ExternalOutput").ap()

    with tile.TileContext(nc) as tc:
        with ExitStack() as ctx:
            _emit(ctx, tc, x_d, wblk_d, mbig_d, bias_d, y_d)
    nc.compile()
    return nc


def make_consts(W, b):
    W = np.asarray(W, dtype=np.float32)
    b = np.asarray(b, dtype=np.float32)
    # W2[c, 5*dx+dy] = W[c, 0, dy, dx]
    W2 = W[:, 0].transpose(0, 2, 1).reshape(25, 25)
    wblk = np.zeros((125, 125), dtype=np.float32)
    for s in range(5):
        wblk[25 * s : 25 * s + 25, 25 * s : 25 * s + 25] = W2.T
    mbig = np.zeros((125, 245), dtype=np.float32)
    for s in range(5):
        mbig[25 * s : 25 * s + 25, 120 + s] = 1.0
    biasv = np.tile(b, 5).astype(np.float32)[:, None]
    # permute from the (s, k) layout to the gather's q = (dy, s, dx) layout
    perm = np.zeros(125, dtype=np.int64)
    for dy in range(5):
        for s in range(5):
            for dx in range(5):
                perm[25 * dy + 5 * s + dx] = 25 * s + 5 * dx + dy
    wblk = wblk[perm][:, perm].astype(ml_dtypes.bfloat16)
    mbig = mbig[perm].astype(ml_dtypes.bfloat16)
    biasv = biasv[perm]
    return wblk, mbig, biasv


_NC_CACHE = None


def get_nc():
    global _NC_CACHE
    if _NC_CACHE is None:
        _NC_CACHE = build()
    return _NC_CACHE


def run(x, W, b, **spmd_kwargs):
    x = np.ascontiguousarray(np.asarray(x, dtype=np.float32))
    wblk, mbig, biasv = make_consts(W, b)
    xs = x.reshape(N_CORES, B_LOC, NPIX)
    in_maps = [
        {"x": xs[c], "wblk": wblk, "mbig": mbig, "biasv": biasv}
        for c in range(N_CORES)
    ]
    nc = get_nc()
    res = bass_utils.run_bass_kernel_spmd(
        nc, in_maps, list(range(N_CORES)), **spmd_kwargs
    )
    y = np.concatenate([res.results[c]["y"] for c in range(N_CORES)], axis=0)
    return y.reshape(B_FULL, 1, 28, 28), res


def kernel(x, W, b):
    y, _ = run(x, W, b)
    return y.astype(np.float32)


# revision 32
# speedup vs baseline: 1.2490x; 1.2490x over previous
"""Dynamic-kernel CNN (conv5x5->tanh gate->windowed sum) on 8 trn2 cores.

out(y,x) = sum_{dx,dy} xq[y+dy, x+dx] * tanh( sum_{k} W2[c,k] V_k + b_c ),
with xq = pad2(x) [32x32], c = k = 5*dx+dy, V_k(y,x) = xq[y+dy, x+dx].

Data-parallel over batch: 2048 images -> 256 per core.

Per-core layout: partitions = (strip s in 0..4) x (channel k in 0..24) = 125.
Strip s of group g handles image 125*r + 5*j + s  (g = 25*r + j).
Free dim = pixel plane (28*28 = 784), stored at row pitch 32.

v3: bf16 compute + three-hop batched im2col gather (few big DMAs).
  DMA instructions have ~2us fixed cost each; the old per-group gather
  (260 DMAs) serialized ~560us on the Sync engine. DMA in-APs may only
  cross partitions in their FIRST level, so the im2col is staged:
  - XQ2 (per r,s: 15 DMAs): XQ2[5r+s, 1024*j + u] = xq[5j+s, 1024r+u]
    -- images of one strip concatenated along the free dim.
  - Z (per r,dy: 15 DMAs): Z[5dy+s, 896*j + t] = XQ2[5r+s, 1024j+32dy+t]
    -- dy-shifted row-windows, groups re-periodized to EXACTLY 896.
  - V (per chunk of 5 groups, ONE DMA): V[25dy+5s+dx, 896j + w] =
    Z[5dy+s, 896j + w + dx]. In-AP [[pitch,25],[1,5],[1,896*csize]]:
    (dy,s) are consecutive partitions and windows are 896-periodic.
    The dx shift bleeds 4 cross-window elements into w in {892..895}
    = (y=27, x>=28) pad columns, which no consumer reads.

Pipeline per group of 5 images:
  1. V [125, 784-of-896] slice of the chunk gather.
  2. FC = blockdiag(W2^T)^T @ V  (two bf16 matmuls into one PSUM tile)
  3. G = tanh(FC + b) on ACT (bias fused, PSUM -> SBUF bf16)
  4. M = V * G elementwise (DVE, bf16)
  5. per-strip channel sum: bf16 matmul with ones-blockdiag placed at
     output partitions 5j..5j+4, accumulating 25 groups into PSUM fp32.
  6. per round of 25 groups: evacuate PSUM -> SBUF -> DMA to output.
"""

import numpy as np
from contextlib import ExitStack

import ml_dtypes

import concourse.bass as bass
import concourse.tile as tile
from concourse import bacc, mybir
from concourse import bass_utils

F32 = mybir.dt.float32
BF16 = mybir.dt.bfloat16
TANH = mybir.ActivationFunctionType.Tanh

N_CORES = 8
B_FULL = 2048
B_LOC = B_FULL // N_CORES  # 256
NPIX = 784                 # 28*28
XQ_LEN = 1024              # 32*32 padded plane
VW = 896                   # 28 rows x 32 cols: per-group window pitch
HALF = 392                 # half pixel plane
CHUNK = 5                  # groups per hop-2 gather DMA
RED_DELAY = 2              # groups of lag for the reduce matmuls

# image -> (round r, group j, strip s): img = 125*r + 5*j + s
ROUNDS = ((0, 25, 125), (1, 25, 125), (2, 2, 6))  # (r, n_groups, rows stored)
MAXG = 25                  # max groups per round


def _emit(ctx, tc, x_d, wblk_d, mbig_d, bias_d, y_d):
    nc = tc.nc

    cpool = ctx.enter_context(tc.tile_pool(name="const", bufs=1))
    zpool = ctx.enter_context(tc.tile_pool(name="z", bufs=2))
    vpool = ctx.enter_context(tc.tile_pool(name="v", bufs=4))
    gpool = ctx.enter_context(tc.tile_pool(name="g", bufs=3))
    mpool = ctx.enter_context(tc.tile_pool(name="m", bufs=4))
    epool = ctx.enter_context(tc.tile_pool(name="evac", bufs=3))
    pfc = ctx.enter_context(tc.tile_pool(name="pfc", bufs=3, space="PSUM"))
    pred = ctx.enter_context(tc.tile_pool(name="pred", bufs=1, space="PSUM"))

    wblk = cpool.tile([125, 125], BF16)
    nc.sync.dma_start(wblk[:], wblk_d[:])
    mbig = cpool.tile([125, 245], BF16)
    nc.sync.dma_start(mbig[:], mbig_d[:])
    biasv = cpool.tile([125, 1], F32)
    nc.sync.dma_start(biasv[:], bias_d[:])

    # padded images: partition p holds xq of image 125*r + p at cols r*1024.
    xq = cpool.tile([128, 3 * XQ_LEN], BF16)
    stage = cpool.tile([128, 3 * NPIX], F32)
    nc.gpsimd.memset(xq[:], 0.0)
    for r, _, rows in ROUNDS:
        nc.sync.dma_start(
            stage[0:rows, NPIX * r : NPIX * (r + 1)],
            x_d[125 * r : 125 * r + rows, :],
        )
        src = stage[0:rows, NPIX * r : NPIX * (r + 1)].rearrange(
            "p (y x) -> p y x", x=28
        )
        dst = xq[0:rows, XQ_LEN * r : XQ_LEN * (r + 1)].rearrange(
            "p (y x) -> p y x", x=32
        )[:, 2:30, 2:30]
        nc.gpsimd.tensor_copy(dst, src)

    xq_ap = xq[:]
    xq_pitch = xq_ap.ap[0][0]  # partition stride in elements

    # --- XQ2[8*(5r+s), 1024*j + u] = xq[5j+s, 1024r + u] ---
    # (r,s) rows spread at partition stride 8 so each lands on its own
    # SBUF AXI port; Z-build reads then fan out over 5 ports, not 2.
    xq2 = cpool.tile([113, XQ_LEN * MAXG], BF16)
    xq2_ap = xq2[:]
    xq2_pitch = xq2_ap.ap[0][0]
    for r, n_groups, rows in ROUNDS:
        for s in range(5):
            p = 8 * (5 * r + s)
            in_ap = bass.AP(
                tensor=xq_ap.tensor,
                offset=xq_ap.offset + xq_pitch * s + XQ_LEN * r,
                ap=[
                    [5 * xq_pitch, n_groups],  # j -> partition step 5
                    [1, XQ_LEN],               # u -> free
                ],
            )
            nc.gpsimd.dma_start(
                xq2[p : p + 1, 0 : XQ_LEN * n_groups], in_ap
            )

    z_tiles = {}
    zdma_eng = (nc.sync, nc.gpsimd)

    def emit_z(ri):
        # --- hop 1: Z[25a+5dy+s, 896*h + t] = XQ2[8(5r+s), 1024(5h+a) +
        # 32dy + t] --- group j = 5h+a lives on partition block 25a (its
        # chunk), so concurrent V-chunk gathers read disjoint port groups.
        r, n_groups, _ = ROUNDS[ri]
        z = zpool.tile([125, VW * CHUNK + 8], BF16, tag="z")
        z_tiles[ri] = z
        seq = 0
        for a in range(min(5, n_groups)):
            nh = (n_groups - a + 4) // 5
            for dy in range(5):
                in_ap = bass.AP(
                    tensor=xq2_ap.tensor,
                    offset=xq2_ap.offset + xq2_pitch * 8 * 5 * r
                    + XQ_LEN * a + 32 * dy,
                    ap=[
                        [8 * xq2_pitch, 5],    # s -> partition (stride 8)
                        [5 * XQ_LEN, nh],      # h -> free block
                        [1, VW],               # t -> free
                    ],
                )
                zdma_eng[seq % 2].dma_start(
                    z[25 * a + 5 * dy : 25 * a + 5 * dy + 5, 0 : VW * nh],
                    in_ap,
                )
                seq += 1

    emit_z(0)
    for ri, (r, n_groups, rows) in enumerate(ROUNDS):
        z = z_tiles[ri]
        z_ap = z[:]
        z_pitch = z_ap.ap[0][0]

        red_a = pred.tile([125, HALF], F32, tag="red_a")
        red_b = pred.tile([125, HALF], F32, tag="red_b")

        # software pipeline: delay each group's reduce matmuls by RED_DELAY
        # groups so the strict-FIFO tensor queue never waits on the
        # ACT->DVE chain of the same group (keeps the PE warm, K=8/8).
        pend = []

        def emit_red(j, m):
            ones_j = mbig[:, 120 - 5 * j : 245 - 5 * j]
            nc.tensor.matmul(
                red_a[:], ones_j, m[:, 0:HALF],
                start=(j == 0), stop=(j == n_groups - 1),
                skip_group_check=True,
            )
            nc.tensor.matmul(
                red_b[:], ones_j, m[:, HALF:NPIX],
                start=(j == 0), stop=(j == n_groups - 1),
                skip_group_check=True,
            )

        vdma_eng = (nc.sync, nc.scalar, nc.gpsimd)
        for a in range(min(5, n_groups)):
            nh = (n_groups - a + 4) // 5
            # --- hop 2: ONE DMA per chunk: V[25dy+5s+dx, 896h + w] =
            # Z[25a+5dy+s, 896h + w + dx]. Chunk a's source is partition
            # block 25a..25a+25, so in-flight chunks hit disjoint ports.
            v = vpool.tile([125, VW * CHUNK], BF16, tag="v")
            in_ap = bass.AP(
                tensor=z_ap.tensor,
                offset=z_ap.offset + z_pitch * 25 * a,
                ap=[
                    [z_pitch, 25],         # (dy,s) -> partition blocks of 5
                    [1, 5],                # dx -> partition
                    [1, VW * nh],          # (h,w) -> free
                ],
            )
            vdma_eng[a % 3].dma_start(v[:, 0 : VW * nh], in_ap)

            # prefetch next round's Z mid-round so the pipeline never drains
            if a == 2 and ri + 1 < len(ROUNDS):
                emit_z(ri + 1)
            elif n_groups <= 2 and a == 0 and ri + 1 < len(ROUNDS):
                emit_z(ri + 1)

            for jj in range(nh):
                j = 5 * jj + a
                # strided views of the real 28x28 pixel plane
                vyx = v[:, VW * jj : VW * (jj + 1)].rearrange(
                    "p (y xc) -> p y xc", xc=32
                )[:, :, 0:28]

                # --- FC matmuls into one 2-bank PSUM tile ---
                fc = pfc.tile([125, 1024], F32)
                nc.tensor.matmul(
                    fc[:, 0:HALF], wblk[:], vyx[:, 0:14], start=True, stop=True
                )
                nc.tensor.matmul(
                    fc[:, 512 : 512 + HALF], wblk[:], vyx[:, 14:28],
                    start=True, stop=True,
                )

                # --- G = tanh(FC + b), one strided ACT over both banks ---
                g = gpool.tile([125, NPIX], BF16)
                fcv = fc[:].rearrange("p (t c) -> p t c", c=512)[:, :, 0:HALF]
                gv = g[:].rearrange("p (t c) -> p t c", c=HALF)
                nc.scalar.activation(gv, fcv, TANH, bias=biasv[:], scale=1.0)

                # --- M = V * G (DVE, 16-bit 2x rate) ---
                m = mpool.tile([125, NPIX], BF16)
                gyx = g[:].rearrange("p (y x) -> p y x", x=28)
                myx = m[:].rearrange("p (y x) -> p y x", x=28)
                nc.vector.tensor_mul(myx, vyx, gyx)

                pend.append((j, m))
                if len(pend) > RED_DELAY:
                    emit_red(*pend.pop(0))

        while pend:
            emit_red(*pend.pop(0))

        # --- evacuate + store round ---
        e_a = epool.tile([125, HALF], F32, tag="e_a")
        nc.vector.tensor_copy(e_a[:], red_a[:])
        nc.sync.dma_start(y_d[125 * r : 125 * r + rows, 0:HALF], e_a[0:rows, :])
        e_b = epool.tile([125, HALF], F32, tag="e_b")
        nc.vector.tensor_copy(e_b[:], red_b[:])
        nc.sync.dma_start(
            y_d[125 * r : 125 * r + rows, HALF:NPIX], e_b[0:rows, :]
        )


def build():
    nc = bacc.Bacc("TRN2", target_bir_lowering=False, debug=False)
    x_d = nc.dram_tensor("x", [B_LOC, NPIX], F32, kind="ExternalInput").ap()
    wblk_d = nc.dram_tensor("wblk", [125, 125], BF16, kind="ExternalInput").ap()
    mbig_d = nc.dram_tensor("mbig", [125, 245], BF16, kind="ExternalInput").ap()
    bias_d = nc.dram_tensor("biasv", [125, 1], F32, kind="ExternalInput").ap()
    y_d = nc.dram_tensor("y", [B_LOC, NPIX], F32, kind="ExternalOutput").ap()

    with tile.TileContext(nc) as tc:
        with ExitStack() as ctx:
            _emit(ctx, tc, x_d, wblk_d, mbig_d, bias_d, y_d)
    nc.compile()
    return nc


def make_consts(W, b):
    W = np.asarray(W, dtype=np.float32)
    b = np.asarray(b, dtype=np.float32)
    # W2[c, 5*dx+dy] = W[c, 0, dy, dx]
    W2 = W[:, 0].transpose(0, 2, 1).reshape(25, 25)
    wblk = np.zeros((125, 125), dtype=np.float32)
    for s in range(5):
        wblk[25 * s : 25 * s + 25, 25 * s : 25 * s + 25] = W2.T
    mbig = np.zeros((125, 245), dtype=np.float32)
    for s in range(5):
        mbig[25 * s : 25 * s + 25, 120 + s] = 1.0
    biasv = np.tile(b, 5).astype(np.float32)[:, None]
    # permute from the (s, k) layout to the gather's q = (dy, s, dx) layout
    perm = np.zeros(125, dtype=np.int64)
    for dy in range(5):
        for s in range(5):
            for dx in range(5):
                perm[25 * dy + 5 * s + dx] = 25 * s + 5 * dx + dy
    wblk = wblk[perm][:, perm].astype(ml_dtypes.bfloat16)
    mbig = mbig[perm].astype(ml_dtypes.bfloat16)
    biasv = biasv[perm]
    return wblk, mbig, biasv


_NC_CACHE = None


def get_nc():
    global _NC_CACHE
    if _NC_CACHE is None:
        _NC_CACHE = build()
    return _NC_CACHE


def run(x, W, b, **spmd_kwargs):
    x = np.ascontiguousarray(np.asarray(x, dtype=np.float32))
    wblk, mbig, biasv = make_consts(W, b)
    xs = x.reshape(N_CORES, B_LOC, NPIX)
    in_maps = [
        {"x": xs[c], "wblk": wblk, "mbig": mbig, "biasv": biasv}
        for c in range(N_CORES)
    ]
    nc = get_nc()
    res = bass_utils.run_bass_kernel_spmd(
        nc, in_maps, list(range(N_CORES)), **spmd_kwargs
    )
    y = np.concatenate([res.results[c]["y"] for c in range(N_CORES)], axis=0)
    return y.reshape(B_FULL, 1, 28, 28), res


def kernel(x, W, b):
    y, _ = run(x, W, b)
    return y.astype(np.float32)
